# revision 8
# baseline (speedup 1.0000x reference)
"""Trainium2 Bass kernel for nn_CombineConcat (pairwise broadcast+concat).

reference semantics (per batch b):
  out[b, i*N + j, 0:D]   = x1[b, i, :]
  out[b, i*N + j, D:2*D] = x2[b, j, :]

Shapes (hardcoded): x1, x2 = [16, 128, 256] f32 -> out = [16, 16384, 512] f32.

Strategy: data-parallel over the batch dim, 2 batches per core on 8 cores.
The kernel is pure data movement and write-bandwidth bound (each core writes
64 MB, reads 256 KB).  Inputs are loaded to SBUF once; the full output is
generated with broadcast (stride-0) DMA reads from SBUF, written straight to
HBM — no compute engines involved.
"""

import numpy as np

_B, _N, _D = 16, 128, 256
_NCORES = 8
_BPC = _B // _NCORES  # batches per core

_NC_CACHE = {}


def _build_nc():
    from kernel_v6 import build_nc

    return build_nc(_BPC, _N, _D, slots_per_batch=16, load_splits=8, G=4)


def _get_nc():
    if "nc" not in _NC_CACHE:
        _NC_CACHE["nc"] = _build_nc()
    return _NC_CACHE["nc"]


def _run(x1, x2, trace=False):
    """Run the kernel on 8 cores; returns (output, BassKernelResults)."""
    from concourse.bass_utils import run_bass_kernel_spmd

    nc = _get_nc()
    x1 = np.ascontiguousarray(np.asarray(x1, dtype=np.float32))
    x2 = np.ascontiguousarray(np.asarray(x2, dtype=np.float32))
    in_maps = [
        {
            "x1": x1[c * _BPC : (c + 1) * _BPC],
            "x2": x2[c * _BPC : (c + 1) * _BPC],
        }
        for c in range(_NCORES)
    ]
    res = run_bass_kernel_spmd(
        nc, in_maps, core_ids=list(range(_NCORES)), trace=trace
    )
    out = np.concatenate([r["out"] for r in res.results], axis=0)
    return out, res


def kernel(x1, x2):
    out, _ = _run(x1, x2, trace=False)
    return out


# revision 9
# speedup vs baseline: 1.0127x; 1.0127x over previous
"""Trainium2 Bass kernel for nn_CombineConcat (pairwise broadcast+concat).

reference semantics (per batch b):
  out[b, i*N + j, 0:D]   = x1[b, i, :]
  out[b, i*N + j, D:2*D] = x2[b, j, :]

Shapes (hardcoded): x1, x2 = [16, 128, 256] f32 -> out = [16, 16384, 512] f32.

Strategy: data-parallel over the batch dim, 2 batches per core on 8 cores.
The kernel is pure data movement and write-bandwidth bound (each core writes
64 MB, reads 256 KB).  Inputs are loaded to SBUF once; the full output is
generated with broadcast (stride-0) DMA reads from SBUF, written straight to
HBM — no compute engines involved.
"""

import numpy as np

_B, _N, _D = 16, 128, 256
_NCORES = 8
_BPC = _B // _NCORES  # batches per core

_NC_CACHE = {}


def _build_nc():
    from kernel_v5 import build_nc

    return build_nc(_BPC, _N, _D, k_ring=24, load_splits=8, G=4)


def _get_nc():
    if "nc" not in _NC_CACHE:
        _NC_CACHE["nc"] = _build_nc()
    return _NC_CACHE["nc"]


def _run(x1, x2, trace=False):
    """Run the kernel on 8 cores; returns (output, BassKernelResults)."""
    from concourse.bass_utils import run_bass_kernel_spmd

    nc = _get_nc()
    x1 = np.ascontiguousarray(np.asarray(x1, dtype=np.float32))
    x2 = np.ascontiguousarray(np.asarray(x2, dtype=np.float32))
    in_maps = [
        {
            "x1": x1[c * _BPC : (c + 1) * _BPC],
            "x2": x2[c * _BPC : (c + 1) * _BPC],
        }
        for c in range(_NCORES)
    ]
    res = run_bass_kernel_spmd(
        nc, in_maps, core_ids=list(range(_NCORES)), trace=trace
    )
    out = np.concatenate([r["out"] for r in res.results], axis=0)
    return out, res


def kernel(x1, x2):
    out, _ = _run(x1, x2, trace=False)
    return out


# revision 10
# speedup vs baseline: 1.1339x; 1.1196x over previous
"""Trainium2 Bass kernel for nn_CombineConcat (pairwise broadcast+concat).

reference semantics (per batch b):
  out[b, i*N + j, 0:D]   = x1[b, i, :]
  out[b, i*N + j, D:2*D] = x2[b, j, :]

Shapes (hardcoded): x1, x2 = [16, 128, 256] f32 -> out = [16, 16384, 512] f32.

Strategy: data-parallel over the batch dim, 2 batches per core on 8 cores.
The kernel is pure data movement and write-bandwidth bound (each core writes
64 MB, reads 256 KB).  Inputs are loaded to SBUF once; the full output is
generated with broadcast (stride-0) DMA reads from SBUF, written straight to
HBM — no compute engines involved.
"""

import numpy as np

_B, _N, _D = 16, 128, 256
_NCORES = 8
_BPC = _B // _NCORES  # batches per core

_NC_CACHE = {}


def _build_nc():
    from kernel_v5 import build_nc

    return build_nc(_BPC, _N, _D, k_ring=16, load_splits=4, G=4)


def _get_nc():
    if "nc" not in _NC_CACHE:
        _NC_CACHE["nc"] = _build_nc()
    return _NC_CACHE["nc"]


def _run(x1, x2, trace=False):
    """Run the kernel on 8 cores; returns (output, BassKernelResults)."""
    from concourse.bass_utils import run_bass_kernel_spmd

    nc = _get_nc()
    x1 = np.ascontiguousarray(np.asarray(x1, dtype=np.float32))
    x2 = np.ascontiguousarray(np.asarray(x2, dtype=np.float32))
    in_maps = [
        {
            "x1": x1[c * _BPC : (c + 1) * _BPC],
            "x2": x2[c * _BPC : (c + 1) * _BPC],
        }
        for c in range(_NCORES)
    ]
    res = run_bass_kernel_spmd(
        nc, in_maps, core_ids=list(range(_NCORES)), trace=trace
    )
    out = np.concatenate([r["out"] for r in res.results], axis=0)
    return out, res


def kernel(x1, x2):
    out, _ = _run(x1, x2, trace=False)
    return out
